# revision 18
# baseline (speedup 1.0000x reference)
"""Baller2VecSeq2Seq forward pass as a Bass/Tile kernel on 8 Trainium2 NeuronCores.

Sharding: sequence-parallel. Each core owns NT=384 of the 3072 tokens.
Weights are replicated. Per attention op, each core computes K/V for its
own tokens and an 8-core AllGather distributes the full K/V; everything
else (QKV projections, scores, softmax, FFN, LayerNorm) is local to the
owned tokens, so there are no AllReduces.

Layout: activations are kept feature-major in SBUF (xT: [512 features on
4x128 partitions, 384 tokens on free dim]).  Every linear is then
``out_T = matmul(lhsT=W_block, rhs=xT_block)`` with fp32 accumulation in
PSUM and no transposes anywhere.  Attention scores are computed with
keys on partitions / queries on free (lhsT=kT tile, rhs=qT tile), exp()
runs on the scalar engine straight out of PSUM (softmax max-subtraction
is skipped; score magnitudes are bounded), and the softmax denominator
falls out of the attn@V matmul via a ones-column appended to V.
LayerNorm statistics are partition-reductions done with ones-vector
matmuls; per-token rows are broadcast back across partitions with
stride-0 DMAs.

Precision: bf16 matmul operands, fp32 accumulation and residual stream.
"""

import math
import os
import sys

import numpy as np
import ml_dtypes

for _p in ("/opt/trn_rl_repo", "/root/.axon_site/_ro/trn_rl_repo"):
    if os.path.isdir(_p) and _p not in sys.path:
        sys.path.append(_p)

from concourse import bass, mybir, bass_utils, bacc  # noqa: E402
import concourse.tile as tile  # noqa: E402
import bass_rust  # noqa: E402
from concourse.vector_clock import ScopedClock  # noqa: E402
from concourse.bass import ts  # noqa: E402

BF16 = ml_dtypes.bfloat16
F32 = mybir.dt.float32
BF = mybir.dt.bfloat16

NCORES = 8
S = 512          # time steps
D = 512          # model dim
NH = 8           # heads
HD = 64          # head dim
DFF = 2048
NLAYERS = 4
NTOK = 6 * S     # 3072 tokens total
NT = NTOK // NCORES   # 384 tokens per core
TT = NT // 128        # 3 token tiles per core
KB = D // 128         # 4 feature blocks
HP = NH // 2          # 4 head pairs
NKT = NTOK // 128     # 24 key tiles
RT = NT // 128        # 3 key tiles per rank
FB = DFF // 128       # 16 ffn blocks
MLP_IN = 35
MLP_HID = 256
NLAB = 121
KSZ = D * NT              # elements of kT_loc in the AG bounce (196608)
VROW = NH * (HD + 1)      # 520: v' row length (per-head 64 cols + ones col)
VSZ = NT * VROW           # 199680
AGSZ = KSZ + VSZ          # 396288 per-rank AllGather payload
RG = [list(range(NCORES))]

Exp = mybir.ActivationFunctionType.Exp
Sqrt = mybir.ActivationFunctionType.Sqrt
Alu = mybir.AluOpType


class TC(tile.TileContext):
    """TileContext whose exit drain spreads sem waits over 1-wait NOPs.

    The walrus CoreV3 CTRL encoding holds only a few sync waits; the stock
    exit drain attaches one wait per outstanding sem and overflows it when
    collectives + several DMA queues are in flight."""

    def _drain_and_barrier(self, tick_clock, wait_clock):
        nc = self.nc
        probe = mybir.InstNoOp(
            name=nc.get_next_instruction_name(),
            text_hint="drain_probe",
            bass_nofuse=True,
        )
        probe.engine = mybir.EngineType.SP
        wait_clock.add_sem_waits(probe, ScopedClock({None: tick_clock.global_clock}))
        si = probe.sync_info
        waits = list(si.on_wait or []) if si is not None else []
        for w in waits:
            nop = nc.sync.nop(nofuse=True)
            sem = bass_rust.SemaphoreHandle(name=w.ant_name, num=w.id)
            bass_rust.wait_op(nop.ins, sem, w.wait_value, "sem-ge", False)
        nc.sync.drain()
        nc.all_engine_barrier()
        assert self.sems is not None
        popped = nc._tile_sem_poison_stack.pop()
        assert popped is self._sem_poison
        nc.clear_and_free_semaphores(list(self.sems.allocated().values()))
        nc.all_engine_barrier()


# ---------------------------------------------------------------------------
# device program
# ---------------------------------------------------------------------------

def _build(stage="full"):
    nc = bacc.Bacc()
    nc.num_devices = NCORES

    di = {}  # dram inputs

    def inp(name, shape, dtype):
        di[name] = nc.dram_tensor(name, shape, dtype, kind="ExternalInput")
        return di[name]

    # per-core tensors
    inp("enc_in", [MLP_IN, NT], BF)
    inp("dec_in", [MLP_IN, NT], BF)
    inp("sel", [1, NT], F32)          # 1.0 on player rows, 0.0 on ball rows
    inp("mask", [NKT, 128, NT], BF)   # additive causal mask (0 / -1e9)
    # frontend weights: [branch, mlp(player|ball), ...]
    inp("fe_w1", [2, 2, MLP_IN, MLP_HID], BF)
    inp("fe_b1", [2, 2, MLP_HID], F32)
    inp("fe_w2", [2, 2, MLP_HID, D], BF)   # pre-scaled by sqrt(D)
    inp("fe_b2", [2, 2, 1, D], BF)         # pre-scaled, used as K=1 lhsT
    # encoder / decoder layer weights
    for pre in ("enc", "dec_sa", "dec_ca"):
        inp(f"{pre}_wq", [NLAYERS, D, D], BF)
        inp(f"{pre}_wk", [NLAYERS, D, D], BF)
        inp(f"{pre}_wv", [NLAYERS, D, D], BF)
        inp(f"{pre}_wo", [NLAYERS, D, D], BF)
        inp(f"{pre}_bq", [NLAYERS, D], F32)
        inp(f"{pre}_bo", [NLAYERS, D], F32)  # bo + bv @ wo folded
    for pre in ("enc", "dec"):
        inp(f"{pre}_w1", [NLAYERS, D, DFF], BF)
        inp(f"{pre}_b1", [NLAYERS, DFF], F32)
        inp(f"{pre}_w2", [NLAYERS, DFF, D], BF)
        inp(f"{pre}_b2", [NLAYERS, D], F32)
    inp("enc_ln", [NLAYERS, 4, D], F32)   # g1,b1,g2,b2
    inp("dec_ln", [NLAYERS, 6, D], F32)   # g1,b1,g2,b2,g3,b3
    inp("cls_w", [D, NLAB], BF)
    inp("cls_b", [NLAB], F32)

    if stage == "full":
        out = nc.dram_tensor("out", [NLAB, NT], F32, kind="ExternalOutput")
    else:
        out = nc.dram_tensor("out", [128, KB, NT], F32, kind="ExternalOutput")

    with tile.TileContext(nc) as tc:
        import contextlib
        ctx = contextlib.ExitStack()
        with ctx:
            _emit(nc, tc, ctx, di, out, stage)
    if not nc.is_finalized():
        nc.finalize()
    return nc


def _emit(nc, tc, ctx, di, out, stage):
    ec = ctx.enter_context
    p_const = ec(tc.tile_pool(name="const", bufs=1))
    p_w = ec(tc.tile_pool(name="weights", bufs=2))
    p_wff = ec(tc.tile_pool(name="ffweights", bufs=2))
    p_act = ec(tc.tile_pool(name="act", bufs=2))
    p_attn = ec(tc.tile_pool(name="attn", bufs=2))
    p_kv = ec(tc.tile_pool(name="kv", bufs=3))
    p_p = ec(tc.tile_pool(name="probs", bufs=3))
    p_small = ec(tc.tile_pool(name="small", bufs=4))
    p_ps = ec(tc.tile_pool(name="psum", bufs=4, space="PSUM"))
    p_ps_o = ec(tc.tile_pool(name="psum_o", bufs=1, space="PSUM"))
    p_ps_m = ec(tc.tile_pool(name="psum_misc", bufs=2, space="PSUM"))
    p_dram = ec(tc.tile_pool(name="dram", bufs=2, space="DRAM"))
    p_dram_sh = ec(tc.tile_pool(name="dram_sh", bufs=2, space="DRAM"))

    ones_col = p_const.tile([128, 1], BF)
    nc.vector.memset(ones_col, 1.0)
    ones_row = p_const.tile([1, 128], F32)
    nc.vector.memset(ones_row, 1.0)
    eps_t = p_const.tile([1, 1], F32)
    nc.vector.memset(eps_t, 1e-5)

    dma = nc.sync.dma_start

    # ---------------- small helpers ----------------

    def bcast_ps(row_ap, parts, tag="bc"):
        """[1, N] fp32 SBUF row -> [parts, N] fp32 SBUF via a DRAM
        round-trip (stride-0 partition reads are only legal from DRAM)."""
        n = row_ap.shape[-1]
        d = p_dram.tile([1, n], F32, tag=f"{tag}_d", bufs=2)
        dma(out=d, in_=row_ap)
        t = p_small.tile([parts, n], F32, tag=tag, bufs=2)
        dma(out=t, in_=d[0:1, :].to_broadcast((parts, n)))
        return t

    def linear_fm(w_sb, x_bf, n_out_blocks, epilogue):
        """Feature-major linear: for each output block m, accumulate over
        input blocks kb: psum += w_sb[:, kb, m*128:...].T @ x_bf[:, kb, :].
        epilogue(m, psum) consumes the [128, NT] psum tile."""
        nkb = w_sb.shape[1]
        for m in range(n_out_blocks):
            ps = p_ps_m.tile([128, NT], F32, tag="mm")
            for kb in range(nkb):
                nc.tensor.matmul(
                    ps,
                    w_sb[:, kb, ts(m, 128)],
                    x_bf[:, kb, :],
                    start=(kb == 0),
                    stop=(kb == nkb - 1),
                )
            epilogue(m, ps)

    def load_w(name, l, shape3, tag):
        """DRAM [l][K, N] -> SBUF [128, K/128, N] (kb-major)."""
        w = p_w.tile(shape3, BF, tag=tag)
        dma(out=w, in_=di[name][l].rearrange("(kb p) n -> p kb n", p=128))
        return w

    def load_wff(name, l, shape3, tag):
        w = p_wff.tile(shape3, BF, tag=tag, bufs=1)
        dma(out=w, in_=di[name][l].rearrange("(kb p) n -> p kb n", p=128))
        return w

    def load_bias(name, l, n, tag):
        """DRAM [l][n*128] -> SBUF [128, n] fp32."""
        b = p_w.tile([128, n], F32, tag=tag)
        dma(out=b, in_=di[name][l].rearrange("(kb p) -> p kb", p=128))
        return b

    def new_xpair(tag):
        x = p_act.tile([128, KB, NT], F32, tag=f"{tag}_f", bufs=1)
        xb = p_act.tile([128, KB, NT], BF, tag=f"{tag}_b", bufs=1)
        return x, xb

    def layernorm(t_f32, g_ap, b_ap, out_tag):
        """Post-norm LN over the feature (partition-block) axis.
        t_f32: [128, KB, NT] fp32. g_ap/b_ap: [128, KB] fp32 SBUF.
        Returns (x_f32, x_bf16)."""
        t_bf = p_small.tile([128, KB, NT], BF, tag="ln_tbf", bufs=1)
        nc.vector.tensor_copy(t_bf, t_f32)
        t2_bf = p_small.tile([128, KB, NT], BF, tag="ln_t2bf", bufs=1)
        nc.vector.tensor_mul(t2_bf, t_f32, t_f32)
        ps0 = p_ps_m.tile([1, NT], F32, tag="mm")
        for kb in range(KB):
            nc.tensor.matmul(ps0, ones_col, t_bf[:, kb, :],
                             start=(kb == 0), stop=(kb == KB - 1))
        ps1 = p_ps_m.tile([1, NT], F32, tag="mm")
        for kb in range(KB):
            nc.tensor.matmul(ps1, ones_col, t2_bf[:, kb, :],
                             start=(kb == 0), stop=(kb == KB - 1))
        mrow = p_small.tile([1, NT], F32, tag="ln_m", bufs=1)
        nc.vector.tensor_scalar_mul(mrow, ps0, 1.0 / D)
        qrow = p_small.tile([1, NT], F32, tag="ln_q", bufs=1)
        nc.vector.tensor_scalar_mul(qrow, ps1, 1.0 / D)
        var = p_small.tile([1, NT], F32, tag="ln_v", bufs=1)
        # var = qrow - mrow^2  ->  (mrow * -mrow) + qrow
        nc.vector.scalar_tensor_tensor(var, mrow, -1.0, mrow, Alu.mult, Alu.mult)
        nc.vector.tensor_add(var, var, qrow)
        sd = p_small.tile([1, NT], F32, tag="ln_sd", bufs=1)
        nc.scalar.activation(sd, var, Sqrt, bias=eps_t[:])
        rs = p_small.tile([1, NT], F32, tag="ln_rs", bufs=1)
        nc.vector.reciprocal(rs, sd)
        mb = bcast_ps(mrow, 128, tag="mm")
        rb = bcast_ps(rs, 128, tag="mm")
        x, xb = new_xpair(out_tag)
        for kb in range(KB):
            tmp = p_small.tile([128, NT], F32, tag="ln_tmp")
            nc.vector.tensor_sub(tmp, t_f32[:, kb, :], mb)
            nc.vector.scalar_tensor_tensor(
                x[:, kb, :], tmp, g_ap[:, kb:kb + 1], rb, Alu.mult, Alu.mult)
            nc.vector.tensor_scalar_add(x[:, kb, :], x[:, kb, :], b_ap[:, kb:kb + 1])
            nc.vector.tensor_copy(xb[:, kb, :], x[:, kb, :])
        return x, xb

    def kv_allgather(x_bf, wk_sb, wv_sb, tag):
        """Compute local kT / v' from x_bf, pack into a bounce buffer and
        AllGather.  Returns the Shared DRAM tile [NCORES, AGSZ]."""
        bounce = p_dram.tile([AGSZ], BF, tag=f"bnc_{tag}")
        kv_all = p_dram_sh.tile([NCORES, AGSZ], BF, addr_space="Shared",
                                tag=f"kva_{tag}")
        # kT [512, 384]
        k_bf = p_attn.tile([128, KB, NT], BF, tag="k_bf", bufs=1)

        def k_epi(m, ps):
            nc.vector.tensor_copy(k_bf[:, m, :], ps)
        linear_fm(wk_sb, x_bf, KB, k_epi)
        dma(out=bounce[0:KSZ].rearrange("(kb p t) -> p kb t", p=128, t=NT),
            in_=k_bf)
        # v' [128, TT, 520] with ones columns
        vp = p_attn.tile([128, TT, VROW], BF, tag="v_bf", bufs=1)
        vp4 = vp.rearrange("p tt (h c) -> p tt h c", c=HD + 1)
        nc.vector.memset(vp4[:, :, :, HD:HD + 1], 1.0)
        for tt in range(TT):
            ps = p_ps_m.tile([128, D], F32, tag="mm")
            for kb in range(KB):
                nc.tensor.matmul(ps, x_bf[:, kb, ts(tt, 128)], wv_sb[:, kb, :],
                                 start=(kb == 0), stop=(kb == KB - 1))
            nc.vector.tensor_copy(
                vp4[:, tt, :, 0:HD],
                ps.rearrange("p (h c) -> p h c", c=HD))
        dma(out=bounce[KSZ:AGSZ].rearrange("(p tt c) -> p tt c", tt=TT, c=VROW),
            in_=vp)
        nc.gpsimd.collective_compute(
            "AllGather", Alu.bypass, replica_groups=RG,
            ins=[bounce[:].opt()], outs=[kv_all[:].opt()])
        return kv_all

    def attention(x_bf, kv_all, q_bf, o_bf, masked):
        """Head-pair streaming attention.  q_bf/o_bf: [128, HP, NT] bf16."""
        if masked:
            msk = p_attn.tile([128, NKT, NT], BF, tag="mask_sb", bufs=1)
            dma(out=msk, in_=di["mask"][:].rearrange("kt p t -> p kt t"))
        for hp in range(HP):
            ps_o0 = p_ps_o.tile([HD + 1, NT], F32, tag="o0")
            ps_o1 = p_ps_o.tile([HD + 1, NT], F32, tag="o1")
            for r in range(NCORES):
                k2 = p_kv.tile([128, NT], BF, tag="k2")
                dma(out=k2,
                    in_=kv_all[r][0:KSZ]
                    .rearrange("(f t) -> f t", t=NT)[ts(hp, 128), :])
                v2 = p_kv.tile([128, RT, 2 * (HD + 1)], BF, tag="v2")
                dma(out=v2,
                    in_=kv_all[r][KSZ:AGSZ]
                    .rearrange("(p tt c) -> p tt c", tt=RT, c=VROW)
                    [:, :, 2 * hp * (HD + 1): 2 * (hp + 1) * (HD + 1)])
                for rtt in range(RT):
                    kt = r * RT + rtt
                    for h01 in range(2):
                        psl = slice(64 * h01, 64 * h01 + 64)
                        ps_s = p_ps.tile([128, NT], F32, tag="s")
                        nc.tensor.matmul(ps_s, k2[psl, ts(rtt, 128)],
                                         q_bf[psl, hp, :], start=True, stop=True)
                        if masked:
                            nc.vector.tensor_add(ps_s, ps_s, msk[:, kt, :])
                        pp = p_p.tile([128, NT], BF, tag="p", bufs=6)
                        nc.scalar.activation(pp, ps_s, Exp, scale=1.0 / math.sqrt(HD))
                        nc.tensor.matmul(
                            ps_o0 if h01 == 0 else ps_o1,
                            v2[:, rtt, ts(h01, HD + 1)], pp,
                            start=(kt == 0), stop=(kt == NKT - 1))
            for h01, ps_o in ((0, ps_o0), (1, ps_o1)):
                rec = p_small.tile([1, NT], F32, tag="rec", bufs=2)
                nc.vector.reciprocal(rec, ps_o[HD:HD + 1, :])
                o_tmp = p_small.tile([HD, NT], F32, tag="o_tmp", bufs=2)
                nc.vector.tensor_copy(o_tmp, ps_o[0:HD, :])
                rb = bcast_ps(rec, HD, tag="mm")
                nc.vector.tensor_mul(o_bf[ts(h01, HD), hp, :], o_tmp, rb)

    def mha(x_f32, x_bf, kv_src_bf, pre, l, masked, out_tag, g_ap, b_ap):
        """Full attention block + residual + LN.  kv_src_bf: activations
        that produce K/V (`x_bf` for self-attn, mem_bf for cross)."""
        wq = load_w(f"{pre}_wq", l, [128, KB, D], "wq")
        wk = load_w(f"{pre}_wk", l, [128, KB, D], "wk")
        wv = load_w(f"{pre}_wv", l, [128, KB, D], "wv")
        wo = load_w(f"{pre}_wo", l, [128, KB, D], "wo")
        bq = load_bias(f"{pre}_bq", l, KB, "bq")
        bo = load_bias(f"{pre}_bo", l, KB, "bo")
        kv_all = kv_allgather(kv_src_bf, wk, wv, f"{pre}{l}")
        q_bf = p_attn.tile([128, HP, NT], BF, tag="q_bf", bufs=1)

        def q_epi(m, ps):
            nc.vector.tensor_scalar_add(q_bf[:, m, :], ps, bq[:, m:m + 1])
        linear_fm(wq, x_bf, HP, q_epi)
        o_bf = p_attn.tile([128, HP, NT], BF, tag="o_bf", bufs=1)
        attention(x_bf, kv_all, q_bf, o_bf, masked)
        t_f32 = p_small.tile([128, KB, NT], F32, tag="resid", bufs=1)

        def o_epi(m, ps):
            nc.vector.scalar_tensor_tensor(
                t_f32[:, m, :], ps, bo[:, m:m + 1], x_f32[:, m, :],
                Alu.add, Alu.add)
        linear_fm(wo, o_bf, KB, o_epi)
        return layernorm(t_f32, g_ap, b_ap, out_tag)

    def ffn(x_f32, x_bf, pre, l, out_tag, g_ap, b_ap):
        w1 = load_wff(f"{pre}_w1", l, [128, KB, DFF], "w1")
        b1 = load_bias(f"{pre}_b1", l, FB, "b1")
        w2 = load_wff(f"{pre}_w2", l, [128, FB, D], "w2")
        b2 = load_bias(f"{pre}_b2", l, KB, "b2")
        h_bf = p_attn.tile([128, FB, NT], BF, tag="h_bf", bufs=1)

        def h_epi(m, ps):
            nc.vector.tensor_scalar(h_bf[:, m, :], ps, b1[:, m:m + 1], 0.0,
                                    Alu.add, Alu.max)
        linear_fm(w1, x_bf, FB, h_epi)
        t_f32 = p_small.tile([128, KB, NT], F32, tag="resid", bufs=1)

        def f_epi(m, ps):
            nc.vector.scalar_tensor_tensor(
                t_f32[:, m, :], ps, b2[:, m:m + 1], x_f32[:, m, :],
                Alu.add, Alu.add)
        linear_fm(w2, h_bf, KB, f_epi)
        return layernorm(t_f32, g_ap, b_ap, out_tag)

    def frontend(br, in_name, sel_bf, isel_bf, selb, iselb, out_tag):
        """Player/ball MLP pair, blended by the sel row."""
        in_sb = p_small.tile([MLP_IN, NT], BF, tag="fe_in", bufs=2)
        dma(out=in_sb, in_=di[in_name][:])
        w1 = p_w.tile([MLP_IN, 2, MLP_HID], BF, tag="fe_w1")
        dma(out=w1, in_=di["fe_w1"][br].rearrange("m k n -> k m n"))
        b1 = p_w.tile([128, 2, 2], F32, tag="fe_b1")
        dma(out=b1, in_=di["fe_b1"][br].rearrange("m (kb p) -> p m kb", p=128))
        w2 = p_w.tile([128, 2, 2, D], BF, tag="fe_w2")
        dma(out=w2, in_=di["fe_w2"][br].rearrange("m (kb p) n -> p m kb n", p=128))
        b2 = p_w.tile([1, 2, D], BF, tag="fe_b2")
        dma(out=b2, in_=di["fe_b2"][br].rearrange("m o n -> o m n"))
        hm = p_attn.tile([128, 2, 2, NT], BF, tag="fe_h", bufs=1)  # [p, mlp, hid-blk, t]
        for mlp in range(2):
            for mt in range(2):
                ps = p_ps_m.tile([128, NT], F32, tag="mm")
                nc.tensor.matmul(ps, w1[:, mlp, ts(mt, 128)], in_sb,
                                 start=True, stop=True)
                # bias + relu, then mask by sel / (1-sel)
                nc.vector.tensor_scalar(ps, ps, b1[:, mlp, mt:mt + 1], 0.0,
                                        Alu.add, Alu.max)
                nc.vector.tensor_mul(hm[:, mlp, mt, :], ps,
                                     selb if mlp == 0 else iselb)
        x, xb = new_xpair(out_tag)
        for m in range(KB):
            ps = p_ps_m.tile([128, NT], F32, tag="mm")
            first = True
            for mlp in range(2):
                for kb in range(2):
                    nc.tensor.matmul(ps, w2[:, mlp, kb, ts(m, 128)],
                                     hm[:, mlp, kb, :], start=first, stop=False)
                    first = False
            nc.tensor.matmul(ps, b2[:, 0, ts(m, 128)], sel_bf,
                             start=False, stop=False)
            nc.tensor.matmul(ps, b2[:, 1, ts(m, 128)], isel_bf,
                             start=False, stop=True)
            nc.vector.tensor_copy(x[:, m, :], ps)
            nc.vector.tensor_copy(xb[:, m, :], x[:, m, :])
        return x, xb

    # ---------------- program ----------------

    # selection rows / broadcasts (shared by both frontends)
    sel_row = p_const.tile([1, NT], F32)
    dma(out=sel_row, in_=di["sel"][:])
    isel_row = p_const.tile([1, NT], F32)
    nc.vector.tensor_scalar(isel_row, sel_row, -1.0, 1.0, Alu.mult, Alu.add)
    sel_bf = p_const.tile([1, NT], BF)
    nc.vector.tensor_copy(sel_bf, sel_row)
    isel_bf = p_const.tile([1, NT], BF)
    nc.vector.tensor_copy(isel_bf, isel_row)
    sel_d = p_dram.tile([2, NT], BF, tag="sel_d")
    dma(out=sel_d[0:1, :], in_=sel_bf)
    dma(out=sel_d[1:2, :], in_=isel_bf)
    selb = p_const.tile([128, NT], BF)
    dma(out=selb, in_=sel_d[0:1, :].to_broadcast((128, NT)))
    iselb = p_const.tile([128, NT], BF)
    dma(out=iselb, in_=sel_d[1:2, :].to_broadcast((128, NT)))

    def ln_g(lsb, i):
        return lsb[:, 2 * i, :]

    def ln_b(lsb, i):
        return lsb[:, 2 * i + 1, :]

    x, xb = frontend(0, "enc_in", sel_bf, isel_bf, selb, iselb, "xe")
    if stage == "fe":
        dma(out=out[:], in_=x)
        return

    if stage == "bc":
        # isolate: fp32 K=1 broadcast matmul + Sqrt/reciprocal rows
        row = p_small.tile([1, NT], F32, tag="ln_m", bufs=1)
        nc.vector.tensor_copy(row, sel_row)
        sd = p_small.tile([1, NT], F32, tag="ln_sd", bufs=1)
        nc.scalar.activation(sd, row, Sqrt, bias=eps_t[:])
        rs = p_small.tile([1, NT], F32, tag="ln_rs", bufs=1)
        nc.vector.reciprocal(rs, sd)
        mb = bcast_ps(rs, 128)
        osb = p_small.tile([128, KB, NT], F32, tag="dbg", bufs=1)
        for kb in range(KB):
            nc.vector.tensor_copy(osb[:, kb, :], mb)
        dma(out=out[:], in_=osb)
        return

    if stage in ("qkv", "att", "attproj", "ln1"):
        wq = load_w("enc_wq", 0, [128, KB, D], "wq")
        wk = load_w("enc_wk", 0, [128, KB, D], "wk")
        wv = load_w("enc_wv", 0, [128, KB, D], "wv")
        bq = load_bias("enc_bq", 0, KB, "bq")
        kv_all = kv_allgather(xb, wk, wv, "dbg")
        q_bf = p_attn.tile([128, HP, NT], BF, tag="q_bf", bufs=1)

        def q_epi(m, ps):
            nc.vector.tensor_scalar_add(q_bf[:, m, :], ps, bq[:, m:m + 1])
        linear_fm(wq, xb, HP, q_epi)
        osb = p_small.tile([128, KB, NT], F32, tag="dbg", bufs=1)
        if stage == "qkv":
            k2 = p_kv.tile([128, NT], BF, tag="k2")
            dma(out=k2,
                in_=kv_all[NCORES - 1][0:KSZ].rearrange("(f t) -> f t", t=NT)[0:128, :])
            for kb in range(KB):
                nc.vector.tensor_copy(osb[:, kb, :], q_bf[:, kb, :])
            nc.vector.tensor_add(osb[:, 0, :], osb[:, 0, :], k2)
        elif stage == "att":
            o_bf = p_attn.tile([128, HP, NT], BF, tag="o_bf", bufs=1)
            attention(xb, kv_all, q_bf, o_bf, False)
            for kb in range(KB):
                nc.vector.tensor_copy(osb[:, kb, :], o_bf[:, kb, :])
        else:
            o_bf = p_attn.tile([128, HP, NT], BF, tag="o_bf", bufs=1)
            attention(xb, kv_all, q_bf, o_bf, False)
            wo = load_w("enc_wo", 0, [128, KB, D], "wo")
            bo = load_bias("enc_bo", 0, KB, "bo")
            t_f32 = p_small.tile([128, KB, NT], F32, tag="resid", bufs=1)

            def o_epi(m, ps):
                nc.vector.scalar_tensor_tensor(
                    t_f32[:, m, :], ps, bo[:, m:m + 1], x[:, m, :],
                    Alu.add, Alu.add)
            linear_fm(wo, o_bf, KB, o_epi)
            if stage == "attproj":
                dma(out=out[:], in_=t_f32)
                return
            lsb = p_w.tile([128, 4, KB], F32, tag="enc_ln")
            dma(out=lsb, in_=di["enc_ln"][0].rearrange("t (kb p) -> p t kb", p=128))
            x2, _x2b = layernorm(t_f32, ln_g(lsb, 0), ln_b(lsb, 0), "xe")
            dma(out=out[:], in_=x2)
            return
        dma(out=out[:], in_=osb)
        return

    n_enc = 1 if stage == "enc1" else NLAYERS
    for l in range(n_enc):
        lsb = p_w.tile([128, 4, KB], F32, tag="enc_ln")
        dma(out=lsb, in_=di["enc_ln"][l].rearrange("t (kb p) -> p t kb", p=128))
        x, xb = mha(x, xb, xb, "enc", l, False, "xe",
                    ln_g(lsb, 0), ln_b(lsb, 0))
        x, xb = ffn(x, xb, "enc", l, "xe", ln_g(lsb, 1), ln_b(lsb, 1))
    if stage in ("enc1", "enc"):
        dma(out=out[:], in_=x)
        return

    mem, mem_bf = x, xb

    # cross-attention K/V for all decoder layers, gathered up front
    kvc = []
    for l in range(NLAYERS):
        wk = load_w("dec_ca_wk", l, [128, KB, D], "wk")
        wv = load_w("dec_ca_wv", l, [128, KB, D], "wv")
        kvc.append(kv_allgather(mem_bf, wk, wv, f"ca{l}"))

    y, yb = frontend(1, "dec_in", sel_bf, isel_bf, selb, iselb, "xd")
    for l in range(NLAYERS):
        lsb = p_w.tile([128, 6, KB], F32, tag="dec_ln")
        dma(out=lsb, in_=di["dec_ln"][l].rearrange("t (kb p) -> p t kb", p=128))
        y, yb = mha(y, yb, yb, "dec_sa", l, True, "xd",
                    ln_g(lsb, 0), ln_b(lsb, 0))
        # cross attention: reuse mha but skip kv computation
        wq = load_w("dec_ca_wq", l, [128, KB, D], "wq")
        wo = load_w("dec_ca_wo", l, [128, KB, D], "wo")
        bq = load_bias("dec_ca_bq", l, KB, "bq")
        bo = load_bias("dec_ca_bo", l, KB, "bo")
        q_bf = p_attn.tile([128, HP, NT], BF, tag="q_bf", bufs=1)

        def q_epi(m, ps, q_bf=q_bf, bq=bq):
            nc.vector.tensor_scalar_add(q_bf[:, m, :], ps, bq[:, m:m + 1])
        linear_fm(wq, yb, HP, q_epi)
        o_bf = p_attn.tile([128, HP, NT], BF, tag="o_bf", bufs=1)
        attention(yb, kvc[l], q_bf, o_bf, False)
        t_f32 = p_small.tile([128, KB, NT], F32, tag="resid", bufs=1)

        def o_epi(m, ps, t_f32=t_f32, bo=bo, y=y):
            nc.vector.scalar_tensor_tensor(
                t_f32[:, m, :], ps, bo[:, m:m + 1], y[:, m, :], Alu.add, Alu.add)
        linear_fm(wo, o_bf, KB, o_epi)
        y, yb = layernorm(t_f32, ln_g(lsb, 1), ln_b(lsb, 1), "xd")
        y, yb = ffn(y, yb, "dec", l, "xd", ln_g(lsb, 2), ln_b(lsb, 2))

    if stage == "dec":
        dma(out=out[:], in_=y)
        return

    # classifier
    wc = p_w.tile([128, KB, NLAB], BF, tag="cls_w")
    dma(out=wc, in_=di["cls_w"][:].rearrange("(kb p) n -> p kb n", p=128))
    bc = p_w.tile([NLAB, 1], F32, tag="cls_b")
    dma(out=bc, in_=di["cls_b"][:].rearrange("(n o) -> n o", o=1))
    ps = p_ps_m.tile([NLAB, NT], F32, tag="mm")
    for kb in range(KB):
        nc.tensor.matmul(ps, wc[:, kb, :], yb[:, kb, :],
                         start=(kb == 0), stop=(kb == KB - 1))
    osb = p_small.tile([NLAB, NT], F32, tag="cls_o")
    nc.vector.tensor_scalar_add(osb, ps, bc)
    dma(out=out[:], in_=osb)


# ---------------------------------------------------------------------------
# host side
# ---------------------------------------------------------------------------

def _np(t, dt=np.float32):
    return np.asarray(t).astype(dt)


def _prep_inputs(params, player_idxs, player_xs, player_ys, player_hoop_sides,
                 ball_xs, ball_ys, ball_zs):
    """Pack weights + frontend features into the kernel's input tensors."""
    f32 = np.float32
    idx = np.asarray(player_idxs).astype(np.int64)
    emb = _np(params["player_emb"])
    bemb = _np(params["ball_emb"])

    def feats_in(start, stop):
        pe = emb[idx[:, start:stop].reshape(-1)]                      # [2560, 32]
        pp = np.concatenate([
            pe,
            _np(player_xs)[:, start:stop].reshape(-1, 1),
            _np(player_ys)[:, start:stop].reshape(-1, 1),
            _np(player_hoop_sides)[:, start:stop].reshape(-1, 1)], 1)
        bpos = np.concatenate([
            np.broadcast_to(bemb, (S, 32)),
            _np(ball_xs)[:, None], _np(ball_ys)[:, None],
            _np(ball_zs)[:, None]], 1)                                # [512, 35]
        return np.concatenate([pp, bpos], 0)                          # [3072, 35]

    shared = {}

    def W(a):
        return _np(a).astype(BF16)

    fe_w1 = np.zeros((2, 2, MLP_IN, MLP_HID), BF16)
    fe_b1 = np.zeros((2, 2, MLP_HID), f32)
    fe_w2 = np.zeros((2, 2, MLP_HID, D), BF16)
    fe_b2 = np.zeros((2, 2, 1, D), BF16)
    for bi, br in enumerate(("enc", "dec")):
        for mi, mlp in enumerate(("player_mlp", "ball_mlp")):
            mp = params[br][mlp]
            fe_w1[bi, mi] = W(mp[0]["w"])
            fe_b1[bi, mi] = _np(mp[0]["b"])
            fe_w2[bi, mi] = W(_np(mp[1]["w"]) * math.sqrt(D))
            fe_b2[bi, mi, 0] = (_np(mp[1]["b"]) * math.sqrt(D)).astype(BF16)
    shared.update(fe_w1=fe_w1, fe_b1=fe_b1, fe_w2=fe_w2, fe_b2=fe_b2)

    def attn_pack(pre, getl):
        for nm in ("wq", "wk", "wv", "wo"):
            shared[f"{pre}_{nm}"] = np.stack(
                [W(getl(l)[nm]) for l in range(NLAYERS)])
        shared[f"{pre}_bq"] = np.stack(
            [_np(getl(l)["bq"]) for l in range(NLAYERS)])
        shared[f"{pre}_bo"] = np.stack(
            [_np(getl(l)["bo"]) + _np(getl(l)["bv"]) @ _np(getl(l)["wo"])
             for l in range(NLAYERS)])

    attn_pack("enc", lambda l: params["enc"]["layers"][l]["sa"])
    attn_pack("dec_sa", lambda l: params["dec"]["layers"][l]["sa"])
    attn_pack("dec_ca", lambda l: params["dec"]["layers"][l]["ca"])
    for pre in ("enc", "dec"):
        lyr = params[pre]["layers"]
        shared[f"{pre}_w1"] = np.stack(
            [W(lyr[l]["ff1"]["w"]) for l in range(NLAYERS)])
        shared[f"{pre}_b1"] = np.stack(
            [_np(lyr[l]["ff1"]["b"]) for l in range(NLAYERS)])
        shared[f"{pre}_w2"] = np.stack(
            [W(lyr[l]["ff2"]["w"]) for l in range(NLAYERS)])
        shared[f"{pre}_b2"] = np.stack(
            [_np(lyr[l]["ff2"]["b"]) for l in range(NLAYERS)])
    enc_ln = np.stack([
        np.stack([_np(params["enc"]["layers"][l][k][c])
                  for k, c in (("ln1", "g"), ("ln1", "b"),
                               ("ln2", "g"), ("ln2", "b"))])
        for l in range(NLAYERS)])
    dec_ln = np.stack([
        np.stack([_np(params["dec"]["layers"][l][k][c])
                  for k, c in (("ln1", "g"), ("ln1", "b"), ("ln2", "g"),
                               ("ln2", "b"), ("ln3", "g"), ("ln3", "b"))])
        for l in range(NLAYERS)])
    shared.update(enc_ln=enc_ln, dec_ln=dec_ln,
                  cls_w=W(params["cls"]["w"]), cls_b=_np(params["cls"]["b"]))

    # causal mask over (key, query-of-core) in key-major tiles
    step = np.concatenate([np.repeat(np.arange(S), 5), np.arange(S)])
    allowed = step[None, :] <= step[:, None]          # [q, k]
    madd = np.where(allowed.T, 0.0, -1e9).astype(BF16)  # [k, q]

    enc_in_full = feats_in(0, 5).T.astype(BF16)   # [35, 3072]
    dec_in_full = feats_in(5, 10).T.astype(BF16)
    sel_full = (np.arange(NTOK) < 5 * S).astype(f32)

    in_maps = []
    for c in range(NCORES):
        sl = slice(c * NT, (c + 1) * NT)
        m = dict(shared)
        m["enc_in"] = np.ascontiguousarray(enc_in_full[:, sl])
        m["dec_in"] = np.ascontiguousarray(dec_in_full[:, sl])
        m["sel"] = sel_full[None, sl].copy()
        m["mask"] = np.ascontiguousarray(
            madd[:, sl].reshape(NKT, 128, NT))
        in_maps.append(m)
    return in_maps


_CACHE = {}


def _get_nc(stage):
    if stage not in _CACHE:
        _CACHE[stage] = _build(stage)
    return _CACHE[stage]


def run(inputs, stage="full", trace=False, tmpdir=None):
    nc = _get_nc(stage)
    in_maps = _prep_inputs(**inputs)
    kw = {}
    if trace:
        try:
            import trace_shim
            trace_shim.install()
        except ImportError:
            pass
        kw = dict(trace=True, tmpdir=tmpdir)
    res = bass_utils.run_bass_kernel_spmd(
        nc, in_maps, core_ids=list(range(NCORES)), **kw)
    outs = [r["out"] for r in res.results]
    if stage == "full":
        full = np.concatenate([o.T for o in outs], 0)  # [3072, 121]
    else:
        # debug stages: out is [128, KB, NT] feature-major -> [NT_tot, D]
        full = np.concatenate(
            [o.transpose(2, 1, 0).reshape(NT, D) for o in outs], 0)
    return full.astype(np.float32), res


def kernel(**inputs):
    out, _ = run(inputs, stage="full")
    return out
